# revision 32
# baseline (speedup 1.0000x reference)
"""AnimeStyleAttention distributed Bass kernel for 8 TRN2 NeuronCores.

Full module: y = (softmax(q k^T / 8) v  *  gate(style)) @ Wo + bo
  with q/k/v = x @ W{q,k,v} + b,  gate = sigmoid(gelu(style@Ws1+bs1)@Ws2+bs2)

Sharding: core c -> (batch b = c//2, head-group g = c%2).  Each core handles
one batch element and 4 of the 8 heads (a 256-channel slice of the QKV space).
Per-batch partial outputs (out_heads * gate) @ Wo_rows are summed pairwise on
the host (plus bo, which only even cores add on device via data, not program).

Layouts on chip (per core, N=2048 tokens, D=512, CH=256, Dh=64):
  xT    4 x [128, 2048] fp16  x^T via hardware DMA-transpose (xbar)
  qT/kT [128, 2, 2048] fp16   (2 head-pairs; partitions 0:64=lo head, 64:128=hi)
  vv    [128, 16, 4, 65] fp16 v plus a ones column per head (65th lhsT column
                              makes attn.v row 64 accumulate the softmax denom)
  scores^T = k @ q^T: [k-tok partitions, q-tok free]; the two heads of a pair
  are row-packed K=64 matmuls (tile_position from base partition) writing one
  2-bank psum tensor, so ONE ScalarE exp call [128, 1024] covers both heads
  (no max subtraction: |scores| <~ 2), psum -> sbuf fp16.
  Per q-chunk tail: denom rows -> fp16 sbuf (DVE), gate-fused psum drain (DVE),
  K=1 ones-matmul broadcasts raw denom across partitions (PE, waits only on the
  tiny row copy), wide full-lane reciprocal (DVE), zT = od * rcb.
  y = zT^T @ Wo + bo interleaved per q-chunk; partial f32 outputs summed
  pairwise on the host.
"""

from contextlib import ExitStack

import numpy as np

import concourse.bacc as bacc
import concourse.bass as bass
import concourse.tile as tile
from concourse import mybir

P = 128
N = 2048          # tokens (one batch element per core)
D = 512           # model dim
CH = 256          # this core's qkv channel slice (4 heads x 64)
NKT = N // P      # 16 token tiles
QC = 512          # q-chunk width
NQC = N // QC     # 4
F16 = mybir.dt.float16
F32 = mybir.dt.float32
AF = mybir.ActivationFunctionType
ALU = mybir.AluOpType


def build_program() -> bass.Bass:
    nc = bacc.Bacc()

    x_d = nc.declare_dram_parameter("x", [N, D], F16, isOutput=False)
    wq_d = nc.declare_dram_parameter("wq", [D, CH], F16, isOutput=False)
    wk_d = nc.declare_dram_parameter("wk", [D, CH], F16, isOutput=False)
    wv_d = nc.declare_dram_parameter("wv", [D, CH], F16, isOutput=False)
    wo_d = nc.declare_dram_parameter("wo", [CH, D], F16, isOutput=False)
    ws1_d = nc.declare_dram_parameter("ws1", [D, D], F16, isOutput=False)
    ws2_d = nc.declare_dram_parameter("ws2", [D, CH], F16, isOutput=False)
    st_d = nc.declare_dram_parameter("style", [D], F16, isOutput=False)
    bq_d = nc.declare_dram_parameter("bq", [CH], F32, isOutput=False)
    bk_d = nc.declare_dram_parameter("bk", [CH], F32, isOutput=False)
    bv_d = nc.declare_dram_parameter("bv", [CH], F32, isOutput=False)
    bs1_d = nc.declare_dram_parameter("bs1", [D], F32, isOutput=False)
    bs2_d = nc.declare_dram_parameter("bs2", [CH], F32, isOutput=False)
    bo_d = nc.declare_dram_parameter("bo", [D], F32, isOutput=False)
    out_d = nc.declare_dram_parameter("out", [N, D], F32, isOutput=True)

    with ExitStack() as ctx:
        tc = ctx.enter_context(tile.TileContext(nc))
        const = ctx.enter_context(tc.tile_pool(name="const", bufs=1))

        # ---- persistent SBUF tensors ----
        xTs = [const.tile([P, N], F16, name=f"xT{i}") for i in range(4)]
        for ci in range(4):
            eng = nc.sync if ci % 2 == 0 else nc.scalar
            eng.dma_start_transpose(xTs[ci], x_d[:, ci * P : (ci + 1) * P])
        wq = const.tile([P, 4, CH], F16)
        nc.gpsimd.dma_start(wq, wq_d.rearrange("(k p) m -> p k m", p=P))
        wk = const.tile([P, 4, CH], F16)
        nc.sync.dma_start(wk, wk_d.rearrange("(k p) m -> p k m", p=P))
        wv = const.tile([P, 4, CH], F16)
        nc.gpsimd.dma_start(wv, wv_d.rearrange("(k p) m -> p k m", p=P))
        wo = const.tile([P, 2, D], F16)
        nc.gpsimd.dma_start(wo, wo_d.rearrange("(k p) m -> p k m", p=P))
        ws1 = const.tile([P, 4, D], F16)
        nc.gpsimd.dma_start(ws1, ws1_d.rearrange("(k p) m -> p k m", p=P))
        ws2 = const.tile([P, 4, CH], F16)
        nc.gpsimd.dma_start(ws2, ws2_d.rearrange("(k p) m -> p k m", p=P))
        stT = const.tile([P, 4], F16)
        nc.sync.dma_start(stT, st_d.rearrange("(c p) -> p c", p=P))
        bqT = const.tile([P, 2], F32)
        nc.sync.dma_start(bqT, bq_d.rearrange("(c p) -> p c", p=P))
        bkT = const.tile([P, 2], F32)
        nc.sync.dma_start(bkT, bk_d.rearrange("(c p) -> p c", p=P))
        bs1T = const.tile([P, 4], F32)
        nc.sync.dma_start(bs1T, bs1_d.rearrange("(c p) -> p c", p=P))
        bs2T = const.tile([P, 2], F32)
        nc.sync.dma_start(bs2T, bs2_d.rearrange("(c p) -> p c", p=P))
        bvb = const.tile([P, CH], F32)
        nc.sync.dma_start(bvb, bv_d.rearrange("(o c) -> o c", o=1).to_broadcast((P, CH)))
        bob = const.tile([P, D], F32)
        nc.sync.dma_start(bob, bo_d.rearrange("(o c) -> o c", o=1).to_broadcast((P, D)))
        ones = const.tile([P, 1], F16)
        nc.vector.memset(ones, 1.0)
        ones_row = const.tile([1, 64], F16)
        nc.vector.memset(ones_row, 1.0)

        qT = const.tile([P, 2, N], F16)
        kT = const.tile([P, 2, N], F16)
        vv = const.tile([P, NKT, 4, 65], F16)  # per head: 64 v cols + ones col
        nc.vector.memset(vv[:, :, :, 64:65], 1.0)
        zT = const.tile([P, 2, N], F16)
        gate = const.tile([P, 2], F32)

        # ---- q/k/v projections ----
        with tc.tile_pool(name="qkps", bufs=4, space="PSUM") as qkps:
            for m in range(2):
                for qc in range(NQC):
                    s = slice(qc * QC, (qc + 1) * QC)
                    psq = qkps.tile([P, QC], F32, tag="ps")
                    for k in range(4):
                        nc.tensor.matmul(
                            psq,
                            lhsT=wq[:, k, m * P : (m + 1) * P],
                            rhs=xTs[k][:, s],
                            start=(k == 0),
                            stop=(k == 3),
                        )
                    nc.vector.tensor_scalar_add(qT[:, m, s], psq, bqT[:, m : m + 1])
                    psk = qkps.tile([P, QC], F32, tag="ps")
                    for k in range(4):
                        nc.tensor.matmul(
                            psk,
                            lhsT=wk[:, k, m * P : (m + 1) * P],
                            rhs=xTs[k][:, s],
                            start=(k == 0),
                            stop=(k == 3),
                        )
                    nc.vector.tensor_scalar_add(kT[:, m, s], psk, bkT[:, m : m + 1])
            for tt in range(NKT):
                psv = qkps.tile([P, CH], F32, tag="ps")
                for k in range(4):
                    nc.tensor.matmul(
                        psv,
                        lhsT=xTs[k][:, tt * P : (tt + 1) * P],
                        rhs=wv[:, k, :],
                        start=(k == 0),
                        stop=(k == 3),
                    )
                nc.vector.tensor_add(
                    vv[:, tt, :, 0:64],
                    psv.rearrange("p (h w) -> p h w", w=64),
                    bvb.rearrange("p (h w) -> p h w", w=64),
                )

        # ---- style gating MLP (tiny) ----
        with (
            tc.tile_pool(name="gps", bufs=1, space="PSUM") as gps,
            tc.tile_pool(name="gsb", bufs=1) as gsb,
        ):
            hps = gps.tile([P, 4], F32)
            for m in range(4):
                for k in range(4):
                    nc.tensor.matmul(
                        hps[:, m : m + 1],
                        lhsT=ws1[:, k, m * P : (m + 1) * P],
                        rhs=stT[:, k : k + 1],
                        start=(k == 0),
                        stop=(k == 3),
                    )
            hT = gsb.tile([P, 4], F16)
            for m in range(4):
                nc.scalar.activation(
                    hT[:, m : m + 1], hps[:, m : m + 1], AF.Gelu,
                    bias=bs1T[:, m : m + 1],
                )
            g_ps = gps.tile([P, 2], F32)
            for m in range(2):
                for k in range(4):
                    nc.tensor.matmul(
                        g_ps[:, m : m + 1],
                        lhsT=ws2[:, k, m * P : (m + 1) * P],
                        rhs=hT[:, k : k + 1],
                        start=(k == 0),
                        stop=(k == 3),
                    )
            for m in range(2):
                nc.scalar.activation(
                    gate[:, m : m + 1], g_ps[:, m : m + 1], AF.Sigmoid,
                    bias=bs2T[:, m : m + 1],
                )

        # ---- attention + fused output projection (q-chunk outer) ----
        with (
            tc.tile_pool(name="scps", bufs=2, space="PSUM") as scps,
            tc.tile_pool(name="ops", bufs=2, space="PSUM") as ops,
            tc.tile_pool(name="rps", bufs=1, space="PSUM") as rps,
            tc.tile_pool(name="yps", bufs=1, space="PSUM") as yps,
            tc.tile_pool(name="esb", bufs=8) as esb,
            tc.tile_pool(name="rsb", bufs=4) as rsb,
            tc.tile_pool(name="ysb", bufs=3) as ysb,
        ):
            for qc in range(NQC):
                s = slice(qc * QC, (qc + 1) * QC)
                for pr in range(2):
                    out_lo = ops.tile([65, QC], F32, tag="o")
                    out_hi = ops.tile([65, QC], F32, tag="o")
                    for kt in range(NKT):
                        ks = slice(kt * P, (kt + 1) * P)
                        # both heads' scores in one 2-bank psum tensor
                        sc = scps.tile([P, 2, QC], F32, tag="sc")
                        nc.tensor.matmul(
                            sc[:, 0, :], lhsT=kT[0:64, pr, ks], rhs=qT[0:64, pr, s],
                            start=True, stop=True,
                        )
                        nc.tensor.matmul(
                            sc[:, 1, :], lhsT=kT[64:128, pr, ks], rhs=qT[64:128, pr, s],
                            start=True, stop=True,
                        )
                        ee = esb.tile([P, 2, QC], F16, tag="e")
                        nc.scalar.activation(ee, sc, AF.Exp)
                        # attn . [v | 1]: row 64 of out accumulates the denominator
                        nc.tensor.matmul(
                            out_lo,
                            lhsT=vv[:, kt, 2 * pr, :],
                            rhs=ee[:, 0, :],
                            start=(kt == 0), stop=(kt == NKT - 1),
                        )
                        nc.tensor.matmul(
                            out_hi,
                            lhsT=vv[:, kt, 2 * pr + 1, :],
                            rhs=ee[:, 1, :],
                            start=(kt == 0), stop=(kt == NKT - 1),
                        )
                    # denom rows to fp16 sbuf first (PE's rcb matmul waits only
                    # on these), then gate-fused psum drains (free the banks)
                    den_l = rsb.tile([1, QC], F16, tag="r")
                    den_h = rsb.tile([1, QC], F16, tag="r")
                    with nc.allow_low_precision(reason="fp16 denom O(1e3)"):
                        nc.vector.tensor_copy(den_l, out_lo[64:65, :])
                        nc.vector.tensor_copy(den_h, out_hi[64:65, :])
                    od = rsb.tile([P, QC], F32, tag="od")
                    nc.vector.tensor_scalar_mul(
                        od[0:64, :], out_lo[0:64, :], gate[0:64, pr : pr + 1]
                    )
                    nc.vector.tensor_scalar_mul(
                        od[64:128, :], out_hi[0:64, :], gate[64:128, pr : pr + 1]
                    )
                    # broadcast raw denom via K=1 ones-matmul, wide recip
                    rcb_ps = rps.tile([P, QC], F32, tag="rb")
                    nc.tensor.matmul(
                        rcb_ps[0:64, :], lhsT=ones_row, rhs=den_l,
                        start=True, stop=True, tile_position=(0, 0),
                    )
                    nc.tensor.matmul(
                        rcb_ps[64:128, :], lhsT=ones_row, rhs=den_h,
                        start=True, stop=True, tile_position=(0, 64),
                    )
                    rcb = rsb.tile([P, QC], F16, tag="rc")
                    with nc.allow_low_precision(reason="fp16 recip of O(1e3) denom"):
                        nc.vector.reciprocal(rcb, rcb_ps)
                    # zT = (attn_out * gate) * (1/den)
                    nc.vector.tensor_mul(zT[:, pr, s], od, rcb)
                # y = zT.T @ Wo + bo for this q-chunk's token tiles
                for tt in range(qc * QC // P, (qc + 1) * QC // P):
                    ps = yps.tile([P, D], F32, tag="y")
                    for m in range(2):
                        nc.tensor.matmul(
                            ps,
                            lhsT=zT[:, m, tt * P : (tt + 1) * P],
                            rhs=wo[:, m, :],
                            start=(m == 0),
                            stop=(m == 1),
                        )
                    y = ysb.tile([P, D], F32, tag="ys")
                    nc.vector.tensor_add(y, ps, bob)
                    nc.sync.dma_start(out_d[tt * P : (tt + 1) * P, :], y)

    nc.finalize()
    return nc


_NC_CACHE = None


def _get_program() -> bass.Bass:
    global _NC_CACHE
    if _NC_CACHE is None:
        _NC_CACHE = build_program()
    return _NC_CACHE


def make_in_maps(inputs: dict) -> list[dict]:
    f16 = np.float16
    f32 = np.float32
    scale = 1.0 / 8.0  # 1/sqrt(head_dim), folded into Wq/bq
    x, style = inputs["x"], inputs["style"]
    in_maps = []
    for c in range(8):
        b, g = divmod(c, 2)
        ch = slice(CH * g, CH * (g + 1))
        in_maps.append(
            {
                "x": np.ascontiguousarray(x[b]).astype(f16),
                "style": np.ascontiguousarray(style[b]).astype(f16),
                "wq": np.ascontiguousarray(inputs["Wq"][:, ch] * scale).astype(f16),
                "wk": np.ascontiguousarray(inputs["Wk"][:, ch]).astype(f16),
                "wv": np.ascontiguousarray(inputs["Wv"][:, ch]).astype(f16),
                "wo": np.ascontiguousarray(inputs["Wo"][ch, :]).astype(f16),
                "ws1": np.ascontiguousarray(inputs["Ws1"]).astype(f16),
                "ws2": np.ascontiguousarray(inputs["Ws2"][:, ch]).astype(f16),
                "bq": np.ascontiguousarray(inputs["bq"][ch] * scale).astype(f32),
                "bk": np.ascontiguousarray(inputs["bk"][ch]).astype(f32),
                "bv": np.ascontiguousarray(inputs["bv"][ch]).astype(f32),
                "bs1": np.ascontiguousarray(inputs["bs1"]).astype(f32),
                "bs2": np.ascontiguousarray(inputs["bs2"][ch]).astype(f32),
                "bo": (
                    np.ascontiguousarray(inputs["bo"]).astype(f32)
                    if g == 0
                    else np.zeros_like(inputs["bo"], dtype=f32)
                ),
            }
        )
    return in_maps


def kernel(**inputs) -> np.ndarray:
    from concourse.bass_utils import run_bass_kernel_spmd

    in_maps = make_in_maps(inputs)
    res = run_bass_kernel_spmd(_get_program(), in_maps, list(range(8))).results
    y = np.stack([res[2 * b]["out"] + res[2 * b + 1]["out"] for b in range(4)])
    return y.astype(np.float32)
